# revision 10
# baseline (speedup 1.0000x reference)
"""Trainium2 Bass kernel for nn_MultiHeadAttention_42271068127395.

Multi-head attention (B=2, T=2048, D=1024, H=16, dk=64) with LoRA on the
QKV projections and an output projection.

Sharding (8 cores): data parallel over batch (2) x tensor parallel over
heads (4 blocks of 4 heads). Each core computes its batch's Q/K/V for its
4 heads, attention, and a partial output projection against its 256-column
block of Wo. The host sums the 4 partials per batch.

Host-side exact rewrites:
  - LoRA folded into weights: W_eff = W + (alpha/r) * B @ A
  - V bias + out bias folded into a final additive row vector.
  - mask is all ones per the input spec, so it is a no-op.
  - all tensors pre-rearranged to [128, ...] partition-major bf16 so every
    DMA lands as contiguous >=2KB per-partition descriptors.

Device strategy (per core), bf16 compute / f32 accumulate:
  - V projection x-stationary -> vaug [t-part, head, dk|ones] directly
    (no PE transposes), one accumulation group per PSUM bank.
  - K/V/Q projections consume x chunks kb-outer, matching DMA arrival.
  - Q projected just-in-time per 512-query block inside attention.
  - scores S^T computed 2 heads concurrently via PE row-tiling
    (tile_position (0,0)/(64,0) auto-derived from base partitions).
  - exp split across engines: pair1 on scalar ACT (true exp); pair0 on
    the vector engine as a one-instruction Schraudolph fast-exp:
    bf16_bits(exp(s/8)) ~= int16(s*(2^7/ln2)/8 + 16248.8), int16 tile
    bitcast to bf16. Scale error cancels in softmax normalization;
    the +-3% sawtooth averages to ~1e-2 on the final output.
  - attn@V emits O^T rows 0-63 + denominator replicated rows 64-127
    (ones columns in vaug; extra PE M is free), lagging scores by 2 tks.
  - boundary work (accumulator copies, normalization, out-projection,
    next Q projection) is spread one-op-per-tk across the next query
    block so neither ACT nor DVE ever stalls the PE's score/exp pipeline.
"""

import os
import sys

for _p in ("/opt/trn_rl_repo", "/root/.axon_site/_ro/trn_rl_repo"):
    if os.path.isdir(_p) and _p not in sys.path:
        sys.path.insert(0, _p)

from contextlib import ExitStack

import numpy as np

import concourse.bass as bass
import concourse.mybir as mybir
import concourse.tile as tile
from concourse import bacc

B = 2
T = 2048
D = 1024
NH = 16
DK = 64
R = 8
ALPHA = 16
SCALING = ALPHA / R

NCORES = 8
HPC = 4            # heads per core
DS = HPC * DK      # 256: per-core slice of the qkv output dim
KB = D // 128      # 8 contraction chunks over D
TB = T // 128      # 16 row tiles of T
QT = T // 512      # 4 query blocks in attention
OB = D // 128      # 8 output row chunks of out projection

F32 = mybir.dt.float32
BF16 = mybir.dt.bfloat16
I16 = mybir.dt.int16
AF = mybir.ActivationFunctionType
ALU = mybir.AluOpType

# Schraudolph fast-exp constants for bf16 bit patterns:
# bits(exp(s/8)) ~= s * (2^7/ln2)/8 + (127*2^7 - C), C tuned zero-mean
EXP_A = (128.0 / float(np.log(2.0))) / 8.0
EXP_B = 16248.8

EXP_MODE = os.environ.get("MHA_EXP", "split")  # split | act


def build_program() -> bass.Bass:
    nc = bacc.Bacc("TRN2", target_bir_lowering=False, debug=False)

    xvr = nc.declare_dram_parameter("xvr", [128, KB * T], BF16, isOutput=False)
    xkr = nc.declare_dram_parameter("xkr", [128, KB * T], BF16, isOutput=False)
    xqr = nc.declare_dram_parameter("xqr", [128, KB * T], BF16, isOutput=False)
    wvr = nc.declare_dram_parameter("wvr", [128, KB * DS], BF16, isOutput=False)
    wkr = nc.declare_dram_parameter("wkr", [128, KB * DS], BF16, isOutput=False)
    wqr = nc.declare_dram_parameter("wqr", [128, KB * DS], BF16, isOutput=False)
    wor = nc.declare_dram_parameter("wor", [128, 2 * D], BF16, isOutput=False)
    bqk = nc.declare_dram_parameter("bqk", [128, 4], F32, isOutput=False)
    outT = nc.declare_dram_parameter("outT", [D, T], F32, isOutput=True)

    with tile.TileContext(nc) as tc, ExitStack() as ctx:
        wp = ctx.enter_context(tc.tile_pool(name="wp", bufs=1))
        pp = ctx.enter_context(tc.tile_pool(name="pp", bufs=7))
        abf = ctx.enter_context(tc.tile_pool(name="abf", bufs=4))
        abh = ctx.enter_context(tc.tile_pool(name="abh", bufs=4))
        od = ctx.enter_context(tc.tile_pool(name="od", bufs=3))
        ps_sc = ctx.enter_context(tc.tile_pool(name="ps_sc", bufs=2, space="PSUM"))
        ps_ac = ctx.enter_context(tc.tile_pool(name="ps_ac", bufs=4, space="PSUM"))

        # ---- persistent SBUF tiles ----
        wv_sb = wp.tile([128, KB, DS], BF16)
        wk_sb = wp.tile([128, KB, DS], BF16)
        wq_sb = wp.tile([128, KB, DS], BF16)
        wo_sb = wp.tile([128, 2, D], BF16)
        bqk_sb = wp.tile([128, 4], F32)
        xv_sb = wp.tile([128, KB, T], BF16)
        xk_sb = wp.tile([128, KB, T], BF16)
        xq_sb = wp.tile([128, KB, T], BF16)
        kt = [wp.tile([128, T], BF16, name=f"kt{i}") for i in range(2)]
        qt = [wp.tile([128, T], BF16, name=f"qt{i}") for i in range(2)]
        # V with ones columns 64-127 per head: attn@V then emits O^T on
        # rows 0-63 and the denominator replicated on rows 64-127.
        vaug = wp.tile([128, TB, HPC, 2 * DK], BF16)

        # ---- DMA issue (3 trigger queues) ----
        # gpsimd queue: weights first, then xq for the JIT Q projections.
        nc.gpsimd.dma_start(
            out=wv_sb, in_=wvr.rearrange("p (a b) -> p a b", a=KB))
        nc.gpsimd.dma_start(
            out=wk_sb, in_=wkr.rearrange("p (a b) -> p a b", a=KB))
        nc.gpsimd.dma_start(out=bqk_sb, in_=bqk[:, :])
        nc.gpsimd.dma_start(
            out=wq_sb, in_=wqr.rearrange("p (a b) -> p a b", a=KB))
        # x chunks split across all three DMA trigger queues; gpsimd (which
        # also carries the weights) takes the tail kbs of each half so the
        # kb-outer consumers never wait on it.
        for which in ("v", "k"):
            sb, dr = (xv_sb, xvr) if which == "v" else (xk_sb, xkr)
            for half in range(2):
                for kb in range(KB):
                    eng = (nc.sync, nc.scalar, nc.scalar, nc.sync,
                           nc.sync, nc.scalar, nc.gpsimd, nc.gpsimd)[kb]
                    eng.dma_start(
                        out=sb[:, kb, half * 1024:(half + 1) * 1024],
                        in_=dr[:, kb * T + half * 1024: kb * T + (half + 1) * 1024])
        # xq for qb0 early (needed by the first JIT Q projection)
        for kb in range(KB):
            eng = (nc.sync, nc.scalar)[kb % 2]
            eng.dma_start(
                out=xq_sb[:, kb, 0:512], in_=xqr[:, kb * T: kb * T + 512])
        nc.gpsimd.dma_start(
            out=wo_sb, in_=wor.rearrange("p (a b) -> p a b", a=2))
        for qb in range(1, QT):
            for kb in range(KB):
                nc.gpsimd.dma_start(
                    out=xq_sb[:, kb, qb * 512:(qb + 1) * 512],
                    in_=xqr[:, kb * T + qb * 512: kb * T + (qb + 1) * 512])

        # warm up the exp table set early so the one-time table load
        # overlaps the projection phase
        warm = wp.tile([1, 1], F32)
        nc.vector.memset(warm, 0.0)
        nc.scalar.activation(warm, warm, AF.Exp)

        # ones columns of vaug (gpsimd, otherwise idle)
        nc.gpsimd.memset(vaug[:, :, :, DK:2 * DK], 1.0)

        # ---- V projection: x-stationary, output already [t, head, dk] ----
        # one accumulation group per PSUM bank (2KB zero-region rule):
        # each 256-wide V t-tile gets its own bank; 8 banks per half-T.
        for half in range(2):
            pv_sc = [
                ps_sc.tile([128, 1024], F32, tag="sc", name=f"pv{half}_{i}")
                for i in range(2)
            ]
            pv_ac = [
                ps_ac.tile([128, 512], F32, tag="ac", name=f"pva{half}_{i}")
                for i in range(4)
            ]

            def pv_slot(t8):
                if t8 < 4:
                    return pv_sc[t8 // 2][:, (t8 % 2) * 512:(t8 % 2) * 512 + 256]
                return pv_ac[t8 - 4][:, 0:256]

            for kb in range(KB):
                for t8 in range(8):
                    tb = half * 8 + t8
                    nc.tensor.matmul(
                        pv_slot(t8),
                        lhsT=xv_sb[:, kb, tb * 128:(tb + 1) * 128],
                        rhs=wv_sb[:, kb],
                        start=(kb == 0),
                        stop=(kb == KB - 1),
                    )
            for t8 in range(8):
                tb = half * 8 + t8
                nc.vector.tensor_copy(
                    vaug[:, tb, :, 0:DK],
                    pv_slot(t8).rearrange("p (h c) -> p h c", h=HPC),
                )

        # ---- K projection -> kt (transposed form), kb-outer rounds so the
        # PE consumes xk chunks in DMA arrival order; bias folded in ----
        for half in range(2):
            pks = [
                ps_ac.tile([128, 512], F32, tag="ac", name=f"pk{half}_{i}")
                for i in range(4)
            ]
            for kb in range(KB):
                for nb2 in range(2):
                    for mb in range(2):
                        csl = slice(half * 1024 + nb2 * 512,
                                    half * 1024 + nb2 * 512 + 512)
                        nc.tensor.matmul(
                            pks[nb2 * 2 + mb],
                            lhsT=wk_sb[:, kb, mb * 128:(mb + 1) * 128],
                            rhs=xk_sb[:, kb, csl],
                            start=(kb == 0),
                            stop=(kb == KB - 1),
                        )
            for nb2 in range(2):
                for mb in range(2):
                    csl = slice(half * 1024 + nb2 * 512,
                                half * 1024 + nb2 * 512 + 512)
                    nc.vector.tensor_scalar_add(
                        kt[mb][:, csl], pks[nb2 * 2 + mb],
                        bqk_sb[:, 2 + mb: 3 + mb])

        def emit_qproj_mms(qb):
            qsl = slice(qb * 512, (qb + 1) * 512)
            psq = ps_sc.tile([128, 1024], F32, tag="sc", name=f"psq{qb}")
            for kb in range(KB):
                for mb in range(2):
                    nc.tensor.matmul(
                        psq[:, mb * 512:(mb + 1) * 512],
                        lhsT=wq_sb[:, kb, mb * 128:(mb + 1) * 128],
                        rhs=xq_sb[:, kb, qsl],
                        start=(kb == 0),
                        stop=(kb == KB - 1),
                    )
            for mb in range(2):
                nc.vector.tensor_scalar_add(
                    qt[mb][:, qsl], psq[:, mb * 512:(mb + 1) * 512],
                    bqk_sb[:, mb: mb + 1])

        emit_qproj_mms(0)

        # ---- attention + out-projection, per query block ----
        # pending = (qb, asbs) where asbs[pair] = (asbO bf16, asbD f32),
        # pair-stacked [2 heads x 64 rows, 512 q].
        pending = None
        for qb in range(QT):
            qsl = slice(qb * 512, (qb + 1) * 512)
            accs = []
            otns = [None, None]
            rcps = [None, None]
            prev_pts = [None, None]  # pts for tk-1, tk-2

            def emit_attnv(tk, pts):
                for h in range(HPC):
                    nc.tensor.matmul(
                        accs[h],
                        lhsT=vaug[:, tk, h, :],
                        rhs=pts[h // 2][:, (h % 2) * 512:(h % 2) * 512 + 512],
                        start=(tk == 0),
                        stop=(tk == TB - 1),
                    )

            for tk in range(TB):
                pts = []
                for pair in range(2):
                    sc = ps_sc.tile(
                        [128, 1024], F32, tag="sc", name=f"sc{qb}_{tk}_{pair}")
                    for hh in range(2):
                        hsl = slice(hh * 64, (hh + 1) * 64)
                        nc.tensor.matmul(
                            sc[:, hh * 512:(hh + 1) * 512],
                            lhsT=kt[pair][hsl, tk * 128:(tk + 1) * 128],
                            rhs=qt[pair][hsl, qsl],
                            start=True,
                            stop=True,
                        )
                    if EXP_MODE != "act" and pair == 0:
                        pt = pp.tile(
                            [128, 1024], I16, tag="pp", name=f"pt{qb}_{tk}_{pair}")
                        nc.vector.tensor_scalar(
                            pt, sc, EXP_A, EXP_B, ALU.mult, ALU.add)
                        pts.append(pt.bitcast(BF16))
                    else:
                        pt = pp.tile(
                            [128, 1024], BF16, tag="pp", name=f"pt{qb}_{tk}_{pair}")
                        nc.scalar.activation(pt, sc, AF.Exp, scale=1.0 / 8.0)
                        pts.append(pt)

                # ---- interleaved boundary work (prev / next block) ----
                if pending is not None:
                    qbP, asbs = pending
                    if tk == 1 or tk == 3:
                        pair = tk // 2
                        rcp = abf.tile(
                            [128, 512], F32, tag="rc", name=f"rcp{qb}_{pair}")
                        nc.vector.reciprocal_approx_fast(
                            out=rcp, in_=asbs[pair][1])
                        rcps[pair] = rcp
                    if tk == 2 or tk == 4:
                        pair = (tk - 1) // 2
                        otn = abh.tile(
                            [128, 512], BF16, tag="ot", name=f"otn{qb}_{pair}")
                        nc.vector.tensor_mul(otn, asbs[pair][0], rcps[pair])
                        otns[pair] = otn
                    if tk in (6, 8, 10, 12):
                        og = (tk - 6) // 2
                        qslP = slice(qbP * 512, (qbP + 1) * 512)
                        po = ps_sc.tile(
                            [128, 2, 512], F32, tag="sc", name=f"po{qbP}_{og}")
                        for j in range(2):
                            ob = og * 2 + j
                            for pair in range(2):
                                nc.tensor.matmul(
                                    po[:, j],
                                    lhsT=wo_sb[:, pair, ob * 128:(ob + 1) * 128],
                                    rhs=otns[pair],
                                    start=(pair == 0),
                                    stop=(pair == 1),
                                )
                        ot = od.tile(
                            [128, 2, 512], F32, tag="od", name=f"ot{qbP}_{og}")
                        nc.scalar.copy(ot, po)
                        nc.sync.dma_start(
                            out=outT.rearrange("(a p) t -> p a t", p=128)[
                                :, og * 2: og * 2 + 2, qslP],
                            in_=ot,
                        )

                if tk == 2:
                    accs.extend(
                        ps_ac.tile([128, 512], F32, tag="ac", name=f"acc{qb}_{h}")
                        for h in range(HPC)
                    )
                if tk >= 2:
                    emit_attnv(tk - 2, prev_pts[1])
                if tk == 13 and qb + 1 < QT:
                    emit_qproj_mms(qb + 1)

                prev_pts = [pts, prev_pts[0]]

            emit_attnv(TB - 2, prev_pts[1])
            emit_attnv(TB - 1, prev_pts[0])

            # copy O^T numerator (bf16, ACT+DVE) and denominator (f32, DVE)
            # out of PSUM; pair-stacked so one recip+mul normalizes a pair
            asbs = []
            for pair in range(2):
                asbO = abh.tile([128, 512], BF16, tag="ot", name=f"aO{qb}_{pair}")
                asbD = abf.tile([128, 512], F32, tag="rc", name=f"aD{qb}_{pair}")
                for hh in range(2):
                    h = pair * 2 + hh
                    psl = slice(hh * 64, (hh + 1) * 64)
                    if hh == 0:
                        nc.scalar.copy(asbO[psl, :], accs[h][0:DK, :])
                    else:
                        nc.vector.tensor_copy(asbO[psl, :], accs[h][0:DK, :])
                    nc.vector.tensor_copy(asbD[psl, :], accs[h][DK:2 * DK, :])
                asbs.append((asbO, asbD))
            pending = (qb, asbs)

        # ---- tail: last block's normalization + out-projection, ordered
        # so pair0's out-proj partial matmuls overlap pair1's norm ----
        qbP, asbs = pending
        qslP = slice(qbP * 512, (qbP + 1) * 512)
        otns = []
        pos = {}

        def tail_po_finish(og, po):
            for j in range(2):
                ob = og * 2 + j
                nc.tensor.matmul(
                    po[:, j], lhsT=wo_sb[:, 1, ob * 128:(ob + 1) * 128],
                    rhs=otns[1], start=False, stop=True)
            ot = od.tile([128, 2, 512], F32, tag="od", name=f"otT_{og}")
            nc.scalar.copy(ot, po)
            nc.sync.dma_start(
                out=outT.rearrange("(a p) t -> p a t", p=128)[
                    :, og * 2: og * 2 + 2, qslP],
                in_=ot,
            )

        for pair in range(2):
            rcp = abf.tile([128, 512], F32, tag="rc", name=f"rcpT_{pair}")
            nc.vector.reciprocal_approx_fast(out=rcp, in_=asbs[pair][1])
            otn = abh.tile([128, 512], BF16, tag="ot", name=f"otnT_{pair}")
            nc.vector.tensor_mul(otn, asbs[pair][0], rcp)
            otns.append(otn)
            if pair == 0:
                for og in range(2):
                    po = ps_sc.tile(
                        [128, 2, 512], F32, tag="sc", name=f"poT_{og}")
                    pos[og] = po
                    for j in range(2):
                        ob = og * 2 + j
                        nc.tensor.matmul(
                            po[:, j],
                            lhsT=wo_sb[:, 0, ob * 128:(ob + 1) * 128],
                            rhs=otns[0], start=True, stop=False)
        for og in range(2):
            tail_po_finish(og, pos[og])
        for og in range(2, 4):
            po = ps_sc.tile([128, 2, 512], F32, tag="sc", name=f"poT_{og}")
            for j in range(2):
                ob = og * 2 + j
                nc.tensor.matmul(
                    po[:, j], lhsT=wo_sb[:, 0, ob * 128:(ob + 1) * 128],
                    rhs=otns[0], start=True, stop=False)
            tail_po_finish(og, po)

    return nc


_NC_CACHE = None


def _get_program():
    global _NC_CACHE
    if _NC_CACHE is None:
        nc = build_program()
        nc.finalize()
        _NC_CACHE = nc
    return _NC_CACHE


def _r128(a):
    """[R, C] -> [128, (R/128)*C] with rows grouped as (chunk, partition)."""
    r, c = a.shape
    n = r // 128
    return np.ascontiguousarray(
        a.reshape(n, 128, c).transpose(1, 0, 2).reshape(128, n * c))


def shard_inputs(
    q, k, v, Wq, bq, Aq, Bq, Wk, bk, Ak, Bk, Wv, bv, Av, Bv, Wo, bo
):
    """Build the 8 per-core input maps."""
    import ml_dtypes

    f = np.float32
    bf = ml_dtypes.bfloat16
    weff = {}
    for name, (W, A, Bm) in {
        "q": (Wq, Aq, Bq),
        "k": (Wk, Ak, Bk),
        "v": (Wv, Av, Bv),
    }.items():
        weff[name] = np.asarray(W, f) + np.float32(SCALING) * (
            np.asarray(Bm, f) @ np.asarray(A, f)
        )

    xr = {}
    for b in range(B):
        xr[("q", b)] = _r128(np.asarray(q, f)[b].T.astype(bf))
        xr[("k", b)] = _r128(np.asarray(k, f)[b].T.astype(bf))
        xr[("v", b)] = _r128(np.asarray(v, f)[b].T.astype(bf))

    in_maps = []
    for c in range(NCORES):
        b = c // 4
        hb = c % 4
        sl = slice(hb * DS, (hb + 1) * DS)
        bqk = np.zeros((128, 4), f)
        bqk[:, 0] = np.asarray(bq, f)[sl][0:128]
        bqk[:, 1] = np.asarray(bq, f)[sl][128:256]
        bqk[:, 2] = np.asarray(bk, f)[sl][0:128]
        bqk[:, 3] = np.asarray(bk, f)[sl][128:256]
        in_maps.append(
            {
                "xqr": xr[("q", b)],
                "xkr": xr[("k", b)],
                "xvr": xr[("v", b)],
                "wqr": _r128(weff["q"][sl].T.astype(bf)),
                "wkr": _r128(weff["k"][sl].T.astype(bf)),
                "wvr": _r128(weff["v"][sl].T.astype(bf)),
                "wor": _r128(np.asarray(Wo, f)[:, sl].T.astype(bf)),
                "bqk": bqk,
            }
        )
    return in_maps


def gather_outputs(results, Wo, bv, bo):
    f = np.float32
    out = np.zeros((B, T, D), f)
    for b in range(B):
        acc = np.zeros((D, T), f)
        for hb in range(4):
            acc += results[b * 4 + hb]["outT"]
        out[b] = acc.T
    out += np.asarray(bv, f) @ np.asarray(Wo, f).T + np.asarray(bo, f)
    return out


def run(inputs: dict, trace: bool = False):
    """Run the sharded kernel; returns (output, BassKernelResults)."""
    from concourse.bass_utils import run_bass_kernel_spmd

    nc = _get_program()
    in_maps = shard_inputs(
        inputs["q"], inputs["k"], inputs["v"],
        inputs["Wq"], inputs["bq"], inputs["Aq"], inputs["Bq"],
        inputs["Wk"], inputs["bk"], inputs["Ak"], inputs["Bk"],
        inputs["Wv"], inputs["bv"], inputs["Av"], inputs["Bv"],
        inputs["Wo"], inputs["bo"],
    )
    br = run_bass_kernel_spmd(nc, in_maps, list(range(NCORES)), trace=trace)
    out = gather_outputs(br.results, inputs["Wo"], inputs["bv"], inputs["bo"])
    return out, br


def kernel(
    q, k, v, mask, Wq, bq, Aq, Bq, Wk, bk, Ak, Bk, Wv, bv, Av, Bv, Wo, bo
):
    inputs = dict(
        q=q, k=k, v=v, mask=mask,
        Wq=Wq, bq=bq, Aq=Aq, Bq=Bq,
        Wk=Wk, bk=bk, Ak=Ak, Bk=Bk,
        Wv=Wv, bv=bv, Av=Av, Bv=Bv,
        Wo=Wo, bo=bo,
    )
    out, _ = run(inputs, trace=False)
    return out


# revision 13
# speedup vs baseline: 1.0185x; 1.0185x over previous
"""Trainium2 Bass kernel for nn_MultiHeadAttention_42271068127395.

Multi-head attention (B=2, T=2048, D=1024, H=16, dk=64) with LoRA on the
QKV projections and an output projection.

Sharding (8 cores): data parallel over batch (2) x tensor parallel over
heads (4 blocks of 4 heads). Each core computes its batch's Q/K/V for its
4 heads, attention, and a partial output projection against its 256-column
block of Wo. The host sums the 4 partials per batch.

Host-side exact rewrites:
  - LoRA folded into weights: W_eff = W + (alpha/r) * B @ A
  - V bias + out bias folded into a final additive row vector.
  - mask is all ones per the input spec, so it is a no-op.
  - all tensors pre-rearranged to [128, ...] partition-major bf16 so every
    DMA lands as contiguous >=2KB per-partition descriptors.

Device strategy (per core), bf16 compute / f32 accumulate:
  - V projection x-stationary -> vaug [t-part, head, dk|ones] directly
    (no PE transposes), one accumulation group per PSUM bank.
  - K/V/Q projections consume x chunks kb-outer, matching DMA arrival.
  - Q projected just-in-time per 512-query block inside attention.
  - scores S^T computed 2 heads concurrently via PE row-tiling
    (tile_position (0,0)/(64,0) auto-derived from base partitions).
  - exp split across engines: pair1 on scalar ACT (true exp); pair0 on
    the vector engine as a one-instruction Schraudolph fast-exp:
    bf16_bits(exp(s/8)) ~= int16(s*(2^7/ln2)/8 + 16248.8), int16 tile
    bitcast to bf16. Scale error cancels in softmax normalization;
    the +-3% sawtooth averages to ~1e-2 on the final output.
  - attn@V emits O^T rows 0-63 + denominator replicated rows 64-127
    (ones columns in vaug; extra PE M is free), lagging scores by 2 tks.
  - boundary work (accumulator copies, normalization, out-projection,
    next Q projection) is spread one-op-per-tk across the next query
    block so neither ACT nor DVE ever stalls the PE's score/exp pipeline.
"""

import os
import sys

for _p in ("/opt/trn_rl_repo", "/root/.axon_site/_ro/trn_rl_repo"):
    if os.path.isdir(_p) and _p not in sys.path:
        sys.path.insert(0, _p)

from contextlib import ExitStack

import numpy as np

import concourse.bass as bass
import concourse.mybir as mybir
import concourse.tile as tile
from concourse import bacc

B = 2
T = 2048
D = 1024
NH = 16
DK = 64
R = 8
ALPHA = 16
SCALING = ALPHA / R

NCORES = 8
HPC = 4            # heads per core
DS = HPC * DK      # 256: per-core slice of the qkv output dim
KB = D // 128      # 8 contraction chunks over D
TB = T // 128      # 16 row tiles of T
QT = T // 512      # 4 query blocks in attention
OB = D // 128      # 8 output row chunks of out projection

F32 = mybir.dt.float32
BF16 = mybir.dt.bfloat16
I16 = mybir.dt.int16
AF = mybir.ActivationFunctionType
ALU = mybir.AluOpType

# Schraudolph fast-exp constants for bf16 bit patterns:
# bits(exp(s/8)) ~= s * (2^7/ln2)/8 + (127*2^7 - C), C tuned zero-mean
EXP_A = (128.0 / float(np.log(2.0))) / 8.0
EXP_B = 16248.8

EXP_MODE = os.environ.get("MHA_EXP", "split")  # split | act


def build_program() -> bass.Bass:
    nc = bacc.Bacc("TRN2", target_bir_lowering=False, debug=False)

    xvr = nc.declare_dram_parameter("xvr", [128, KB * T], BF16, isOutput=False)
    xkr = nc.declare_dram_parameter("xkr", [128, KB * T], BF16, isOutput=False)
    xqr = nc.declare_dram_parameter("xqr", [128, KB * T], BF16, isOutput=False)
    wvr = nc.declare_dram_parameter("wvr", [128, KB * DS], BF16, isOutput=False)
    wkr = nc.declare_dram_parameter("wkr", [128, KB * DS], BF16, isOutput=False)
    wqr = nc.declare_dram_parameter("wqr", [128, KB * DS], BF16, isOutput=False)
    wor = nc.declare_dram_parameter("wor", [128, 2 * D], BF16, isOutput=False)
    bqk = nc.declare_dram_parameter("bqk", [128, 4], F32, isOutput=False)
    outT = nc.declare_dram_parameter("outT", [D, T], F32, isOutput=True)

    with tile.TileContext(nc) as tc, ExitStack() as ctx:
        wp = ctx.enter_context(tc.tile_pool(name="wp", bufs=1))
        pp = ctx.enter_context(tc.tile_pool(name="pp", bufs=7))
        abf = ctx.enter_context(tc.tile_pool(name="abf", bufs=4))
        abh = ctx.enter_context(tc.tile_pool(name="abh", bufs=4))
        od = ctx.enter_context(tc.tile_pool(name="od", bufs=3))
        ps_sc = ctx.enter_context(tc.tile_pool(name="ps_sc", bufs=2, space="PSUM"))
        ps_ac = ctx.enter_context(tc.tile_pool(name="ps_ac", bufs=4, space="PSUM"))

        # ---- persistent SBUF tiles ----
        wv_sb = wp.tile([128, KB, DS], BF16)
        wk_sb = wp.tile([128, KB, DS], BF16)
        wq_sb = wp.tile([128, KB, DS], BF16)
        wo_sb = wp.tile([128, 2, D], BF16)
        bqk_sb = wp.tile([128, 4], F32)
        xv_sb = wp.tile([128, KB, T], BF16)
        xk_sb = wp.tile([128, KB, T], BF16)
        xq_sb = wp.tile([128, KB, T], BF16)
        kt = [wp.tile([128, T], BF16, name=f"kt{i}") for i in range(2)]
        qt = [wp.tile([128, T], BF16, name=f"qt{i}") for i in range(2)]
        # V with ones columns 64-127 per head: attn@V then emits O^T on
        # rows 0-63 and the denominator replicated on rows 64-127.
        vaug = wp.tile([128, TB, HPC, 2 * DK], BF16)

        # ---- DMA issue (3 trigger queues) ----
        # gpsimd queue: weights first, then xq for the JIT Q projections.
        nc.gpsimd.dma_start(
            out=wv_sb, in_=wvr.rearrange("p (a b) -> p a b", a=KB))
        nc.gpsimd.dma_start(
            out=wk_sb, in_=wkr.rearrange("p (a b) -> p a b", a=KB))
        nc.gpsimd.dma_start(out=bqk_sb, in_=bqk[:, :])
        nc.gpsimd.dma_start(
            out=wq_sb, in_=wqr.rearrange("p (a b) -> p a b", a=KB))
        # x chunks on sync/scalar in consumption order: half-0 of v and k,
        # then qb0's q slice, then the second halves (which are consumed by
        # projection work injected into attention block 0).
        def x_chunk(sb, dr, kb, c0, c1):
            eng = nc.sync if kb % 2 == 0 else nc.scalar
            eng.dma_start(
                out=sb[:, kb, c0:c1], in_=dr[:, kb * T + c0: kb * T + c1])

        for kb in range(KB):
            x_chunk(xv_sb, xvr, kb, 0, 1024)
        for kb in range(KB):
            x_chunk(xk_sb, xkr, kb, 0, 1024)
        for kb in range(KB):
            x_chunk(xq_sb, xqr, kb, 0, 512)
        for kb in range(KB):
            x_chunk(xk_sb, xkr, kb, 1024, 2048)
        for kb in range(KB):
            x_chunk(xv_sb, xvr, kb, 1024, 2048)
        nc.gpsimd.dma_start(
            out=wo_sb, in_=wor.rearrange("p (a b) -> p a b", a=2))
        for qb in range(1, QT):
            for kb in range(KB):
                nc.gpsimd.dma_start(
                    out=xq_sb[:, kb, qb * 512:(qb + 1) * 512],
                    in_=xqr[:, kb * T + qb * 512: kb * T + (qb + 1) * 512])

        # warm up the exp table set early so the one-time table load
        # overlaps the projection phase
        warm = wp.tile([1, 1], F32)
        nc.vector.memset(warm, 0.0)
        nc.scalar.activation(warm, warm, AF.Exp)

        # ones columns of vaug (gpsimd, otherwise idle)
        nc.gpsimd.memset(vaug[:, :, :, DK:2 * DK], 1.0)

        # ---- V projection half-0: x-stationary, output [t, head, dk] ----
        # one accumulation group per PSUM bank (2KB zero-region rule):
        # each 256-wide V t-tile gets its own bank; 8 banks for half-T.
        pv_sc = [
            ps_sc.tile([128, 1024], F32, tag="sc", name=f"pv0_{i}")
            for i in range(2)
        ]
        pv_ac = [
            ps_ac.tile([128, 512], F32, tag="ac", name=f"pva0_{i}")
            for i in range(4)
        ]

        def pv_slot(t8):
            if t8 < 4:
                return pv_sc[t8 // 2][:, (t8 % 2) * 512:(t8 % 2) * 512 + 256]
            return pv_ac[t8 - 4][:, 0:256]

        for kb in range(KB):
            for t8 in range(8):
                nc.tensor.matmul(
                    pv_slot(t8),
                    lhsT=xv_sb[:, kb, t8 * 128:(t8 + 1) * 128],
                    rhs=wv_sb[:, kb],
                    start=(kb == 0),
                    stop=(kb == KB - 1),
                )
        for t8 in range(8):
            nc.vector.tensor_copy(
                vaug[:, t8, :, 0:DK],
                pv_slot(t8).rearrange("p (h c) -> p h c", h=HPC),
            )

        # ---- K projection half-0 -> kt, kb-outer so the PE consumes xk
        # chunks in DMA arrival order; bias folded in ----
        pks = [
            ps_ac.tile([128, 512], F32, tag="ac", name=f"pk0_{i}")
            for i in range(4)
        ]
        for kb in range(KB):
            for nb2 in range(2):
                for mb in range(2):
                    csl = slice(nb2 * 512, nb2 * 512 + 512)
                    nc.tensor.matmul(
                        pks[nb2 * 2 + mb],
                        lhsT=wk_sb[:, kb, mb * 128:(mb + 1) * 128],
                        rhs=xk_sb[:, kb, csl],
                        start=(kb == 0),
                        stop=(kb == KB - 1),
                    )
        for nb2 in range(2):
            for mb in range(2):
                csl = slice(nb2 * 512, nb2 * 512 + 512)
                nc.vector.tensor_scalar_add(
                    kt[mb][:, csl], pks[nb2 * 2 + mb],
                    bqk_sb[:, 2 + mb: 3 + mb])

        # ---- half-1 K/V projections, injected into attention block 0
        # using only the scores-ring PSUM banks (accs own the rest) ----
        def emit_kproj_half1():
            for r in range(2):
                pk = ps_sc.tile([128, 1024], F32, tag="sc", name=f"pkB{r}")
                for kb in range(KB):
                    for i in range(2):
                        g = r * 2 + i
                        nb2, mb = g // 2, g % 2
                        csl = slice(1024 + nb2 * 512, 1024 + nb2 * 512 + 512)
                        nc.tensor.matmul(
                            pk[:, i * 512:(i + 1) * 512],
                            lhsT=wk_sb[:, kb, mb * 128:(mb + 1) * 128],
                            rhs=xk_sb[:, kb, csl],
                            start=(kb == 0),
                            stop=(kb == KB - 1),
                        )
                for i in range(2):
                    g = r * 2 + i
                    nb2, mb = g // 2, g % 2
                    csl = slice(1024 + nb2 * 512, 1024 + nb2 * 512 + 512)
                    nc.vector.tensor_scalar_add(
                        kt[mb][:, csl], pk[:, i * 512:(i + 1) * 512],
                        bqk_sb[:, 2 + mb: 3 + mb])

        def emit_vproj_half1():
            for g in range(4):
                pv = ps_sc.tile([128, 1024], F32, tag="sc", name=f"pvB{g}")
                for kb in range(KB):
                    for i in range(2):
                        tb = 8 + g * 2 + i
                        nc.tensor.matmul(
                            pv[:, i * 512: i * 512 + 256],
                            lhsT=xv_sb[:, kb, tb * 128:(tb + 1) * 128],
                            rhs=wv_sb[:, kb],
                            start=(kb == 0),
                            stop=(kb == KB - 1),
                        )
                for i in range(2):
                    tb = 8 + g * 2 + i
                    nc.vector.tensor_copy(
                        vaug[:, tb, :, 0:DK],
                        pv[:, i * 512: i * 512 + 256].rearrange(
                            "p (h c) -> p h c", h=HPC),
                    )

        def emit_qproj_mms(qb):
            qsl = slice(qb * 512, (qb + 1) * 512)
            psq = ps_sc.tile([128, 1024], F32, tag="sc", name=f"psq{qb}")
            for kb in range(KB):
                for mb in range(2):
                    nc.tensor.matmul(
                        psq[:, mb * 512:(mb + 1) * 512],
                        lhsT=wq_sb[:, kb, mb * 128:(mb + 1) * 128],
                        rhs=xq_sb[:, kb, qsl],
                        start=(kb == 0),
                        stop=(kb == KB - 1),
                    )
            for mb in range(2):
                nc.vector.tensor_scalar_add(
                    qt[mb][:, qsl], psq[:, mb * 512:(mb + 1) * 512],
                    bqk_sb[:, mb: mb + 1])

        emit_qproj_mms(0)

        # ---- attention + out-projection, per query block ----
        # pending = (qb, asbs) where asbs[pair] = (asbO bf16, asbD f32),
        # pair-stacked [2 heads x 64 rows, 512 q].
        pending = None
        for qb in range(QT):
            qsl = slice(qb * 512, (qb + 1) * 512)
            accs = []
            otns = [None, None]
            rcps = [None, None]
            prev_pts = [None, None]  # pts for tk-1, tk-2

            def emit_attnv(tk, pts):
                for h in range(HPC):
                    nc.tensor.matmul(
                        accs[h],
                        lhsT=vaug[:, tk, h, :],
                        rhs=pts[h // 2][:, (h % 2) * 512:(h % 2) * 512 + 512],
                        start=(tk == 0),
                        stop=(tk == TB - 1),
                    )

            for tk in range(TB):
                pts = []
                for pair in range(2):
                    sc = ps_sc.tile(
                        [128, 1024], F32, tag="sc", name=f"sc{qb}_{tk}_{pair}")
                    for hh in range(2):
                        hsl = slice(hh * 64, (hh + 1) * 64)
                        nc.tensor.matmul(
                            sc[:, hh * 512:(hh + 1) * 512],
                            lhsT=kt[pair][hsl, tk * 128:(tk + 1) * 128],
                            rhs=qt[pair][hsl, qsl],
                            start=True,
                            stop=True,
                        )
                    if EXP_MODE != "act" and pair == 0:
                        pt = pp.tile(
                            [128, 1024], I16, tag="pp", name=f"pt{qb}_{tk}_{pair}")
                        nc.vector.tensor_scalar(
                            pt, sc, EXP_A, EXP_B, ALU.mult, ALU.add)
                        pts.append(pt.bitcast(BF16))
                    else:
                        pt = pp.tile(
                            [128, 1024], BF16, tag="pp", name=f"pt{qb}_{tk}_{pair}")
                        nc.scalar.activation(pt, sc, AF.Exp, scale=1.0 / 8.0)
                        pts.append(pt)

                # ---- interleaved boundary work (prev / next block) ----
                if pending is not None:
                    qbP, asbs = pending
                    if tk == 1 or tk == 3:
                        pair = tk // 2
                        rcp = abf.tile(
                            [128, 512], F32, tag="rc", name=f"rcp{qb}_{pair}")
                        nc.vector.reciprocal_approx_fast(
                            out=rcp, in_=asbs[pair][1])
                        rcps[pair] = rcp
                    if tk == 2 or tk == 4:
                        pair = (tk - 1) // 2
                        otn = abh.tile(
                            [128, 512], BF16, tag="ot", name=f"otn{qb}_{pair}")
                        nc.vector.tensor_mul(otn, asbs[pair][0], rcps[pair])
                        otns[pair] = otn
                    if tk in (6, 8, 10, 12):
                        og = (tk - 6) // 2
                        qslP = slice(qbP * 512, (qbP + 1) * 512)
                        po = ps_sc.tile(
                            [128, 2, 512], F32, tag="sc", name=f"po{qbP}_{og}")
                        for j in range(2):
                            ob = og * 2 + j
                            for pair in range(2):
                                nc.tensor.matmul(
                                    po[:, j],
                                    lhsT=wo_sb[:, pair, ob * 128:(ob + 1) * 128],
                                    rhs=otns[pair],
                                    start=(pair == 0),
                                    stop=(pair == 1),
                                )
                        ot = od.tile(
                            [128, 2, 512], F32, tag="od", name=f"ot{qbP}_{og}")
                        nc.scalar.copy(ot, po)
                        nc.sync.dma_start(
                            out=outT.rearrange("(a p) t -> p a t", p=128)[
                                :, og * 2: og * 2 + 2, qslP],
                            in_=ot,
                        )

                if tk == 2:
                    accs.extend(
                        ps_ac.tile([128, 512], F32, tag="ac", name=f"acc{qb}_{h}")
                        for h in range(HPC)
                    )
                if tk >= 2:
                    emit_attnv(tk - 2, prev_pts[1])
                if qb == 0 and tk == 7:
                    # second-half K/V projections ride inside block 0's
                    # PE stream, overlapping their own DMA arrival;
                    # scores(tk8)/attnV(tk8) depend on their outputs
                    emit_kproj_half1()
                    emit_vproj_half1()
                if tk == 13 and qb + 1 < QT:
                    emit_qproj_mms(qb + 1)

                prev_pts = [pts, prev_pts[0]]

            emit_attnv(TB - 2, prev_pts[1])
            emit_attnv(TB - 1, prev_pts[0])

            # copy O^T numerator (bf16, ACT+DVE) and denominator (f32, DVE)
            # out of PSUM; pair-stacked so one recip+mul normalizes a pair
            asbs = []
            for pair in range(2):
                asbO = abh.tile([128, 512], BF16, tag="ot", name=f"aO{qb}_{pair}")
                asbD = abf.tile([128, 512], F32, tag="rc", name=f"aD{qb}_{pair}")
                for hh in range(2):
                    h = pair * 2 + hh
                    psl = slice(hh * 64, (hh + 1) * 64)
                    if hh == 0:
                        nc.scalar.copy(asbO[psl, :], accs[h][0:DK, :])
                    else:
                        nc.vector.tensor_copy(asbO[psl, :], accs[h][0:DK, :])
                    nc.vector.tensor_copy(asbD[psl, :], accs[h][DK:2 * DK, :])
                asbs.append((asbO, asbD))
            pending = (qb, asbs)

        # ---- tail: last block's normalization + out-projection, ordered
        # so pair0's out-proj partial matmuls overlap pair1's norm ----
        qbP, asbs = pending
        qslP = slice(qbP * 512, (qbP + 1) * 512)
        otns = []
        pos = {}

        def tail_po_finish(og, po):
            for j in range(2):
                ob = og * 2 + j
                nc.tensor.matmul(
                    po[:, j], lhsT=wo_sb[:, 1, ob * 128:(ob + 1) * 128],
                    rhs=otns[1], start=False, stop=True)
            ot = od.tile([128, 2, 512], F32, tag="od", name=f"otT_{og}")
            nc.scalar.copy(ot, po)
            nc.sync.dma_start(
                out=outT.rearrange("(a p) t -> p a t", p=128)[
                    :, og * 2: og * 2 + 2, qslP],
                in_=ot,
            )

        for pair in range(2):
            rcp = abf.tile([128, 512], F32, tag="rc", name=f"rcpT_{pair}")
            nc.vector.reciprocal_approx_fast(out=rcp, in_=asbs[pair][1])
            otn = abh.tile([128, 512], BF16, tag="ot", name=f"otnT_{pair}")
            nc.vector.tensor_mul(otn, asbs[pair][0], rcp)
            otns.append(otn)
            if pair == 0:
                for og in range(2):
                    po = ps_sc.tile(
                        [128, 2, 512], F32, tag="sc", name=f"poT_{og}")
                    pos[og] = po
                    for j in range(2):
                        ob = og * 2 + j
                        nc.tensor.matmul(
                            po[:, j],
                            lhsT=wo_sb[:, 0, ob * 128:(ob + 1) * 128],
                            rhs=otns[0], start=True, stop=False)
        for og in range(2):
            tail_po_finish(og, pos[og])
        for og in range(2, 4):
            po = ps_sc.tile([128, 2, 512], F32, tag="sc", name=f"poT_{og}")
            for j in range(2):
                ob = og * 2 + j
                nc.tensor.matmul(
                    po[:, j], lhsT=wo_sb[:, 0, ob * 128:(ob + 1) * 128],
                    rhs=otns[0], start=True, stop=False)
            tail_po_finish(og, po)

    return nc


_NC_CACHE = None


def _get_program():
    global _NC_CACHE
    if _NC_CACHE is None:
        nc = build_program()
        nc.finalize()
        _NC_CACHE = nc
    return _NC_CACHE


def _r128(a):
    """[R, C] -> [128, (R/128)*C] with rows grouped as (chunk, partition)."""
    r, c = a.shape
    n = r // 128
    return np.ascontiguousarray(
        a.reshape(n, 128, c).transpose(1, 0, 2).reshape(128, n * c))


def shard_inputs(
    q, k, v, Wq, bq, Aq, Bq, Wk, bk, Ak, Bk, Wv, bv, Av, Bv, Wo, bo
):
    """Build the 8 per-core input maps."""
    import ml_dtypes

    f = np.float32
    bf = ml_dtypes.bfloat16
    weff = {}
    for name, (W, A, Bm) in {
        "q": (Wq, Aq, Bq),
        "k": (Wk, Ak, Bk),
        "v": (Wv, Av, Bv),
    }.items():
        weff[name] = np.asarray(W, f) + np.float32(SCALING) * (
            np.asarray(Bm, f) @ np.asarray(A, f)
        )

    xr = {}
    for b in range(B):
        xr[("q", b)] = _r128(np.asarray(q, f)[b].T.astype(bf))
        xr[("k", b)] = _r128(np.asarray(k, f)[b].T.astype(bf))
        xr[("v", b)] = _r128(np.asarray(v, f)[b].T.astype(bf))

    in_maps = []
    for c in range(NCORES):
        b = c // 4
        hb = c % 4
        sl = slice(hb * DS, (hb + 1) * DS)
        bqk = np.zeros((128, 4), f)
        bqk[:, 0] = np.asarray(bq, f)[sl][0:128]
        bqk[:, 1] = np.asarray(bq, f)[sl][128:256]
        bqk[:, 2] = np.asarray(bk, f)[sl][0:128]
        bqk[:, 3] = np.asarray(bk, f)[sl][128:256]
        in_maps.append(
            {
                "xqr": xr[("q", b)],
                "xkr": xr[("k", b)],
                "xvr": xr[("v", b)],
                "wqr": _r128(weff["q"][sl].T.astype(bf)),
                "wkr": _r128(weff["k"][sl].T.astype(bf)),
                "wvr": _r128(weff["v"][sl].T.astype(bf)),
                "wor": _r128(np.asarray(Wo, f)[:, sl].T.astype(bf)),
                "bqk": bqk,
            }
        )
    return in_maps


def gather_outputs(results, Wo, bv, bo):
    f = np.float32
    out = np.zeros((B, T, D), f)
    for b in range(B):
        acc = np.zeros((D, T), f)
        for hb in range(4):
            acc += results[b * 4 + hb]["outT"]
        out[b] = acc.T
    out += np.asarray(bv, f) @ np.asarray(Wo, f).T + np.asarray(bo, f)
    return out


def run(inputs: dict, trace: bool = False):
    """Run the sharded kernel; returns (output, BassKernelResults)."""
    from concourse.bass_utils import run_bass_kernel_spmd

    nc = _get_program()
    in_maps = shard_inputs(
        inputs["q"], inputs["k"], inputs["v"],
        inputs["Wq"], inputs["bq"], inputs["Aq"], inputs["Bq"],
        inputs["Wk"], inputs["bk"], inputs["Ak"], inputs["Bk"],
        inputs["Wv"], inputs["bv"], inputs["Av"], inputs["Bv"],
        inputs["Wo"], inputs["bo"],
    )
    br = run_bass_kernel_spmd(nc, in_maps, list(range(NCORES)), trace=trace)
    out = gather_outputs(br.results, inputs["Wo"], inputs["bv"], inputs["bo"])
    return out, br


def kernel(
    q, k, v, mask, Wq, bq, Aq, Bq, Wk, bk, Ak, Bk, Wv, bv, Av, Bv, Wo, bo
):
    inputs = dict(
        q=q, k=k, v=v, mask=mask,
        Wq=Wq, bq=bq, Aq=Aq, Bq=Bq,
        Wk=Wk, bk=bk, Ak=Ak, Bk=Bk,
        Wv=Wv, bv=bv, Av=Av, Bv=Bv,
        Wo=Wo, bo=bo,
    )
    out, _ = run(inputs, trace=False)
    return out
